# revision 37
# baseline (speedup 1.0000x reference)
"""Trainium2 Bass kernel for nn_BilinAndFwdComboVecComp.

Math (B=8, S=256, C=256, V=64):
  final[b,s,z,k] = tanh( sum_ij ctx[b,s,i] ctx[b,z,j] W'[i,j,k] + A[b,z,k] + Bt[b,s,k] )
where
  W'[i,j,k] = W[i,j,k] + (i==j) * linmul_w[k,i]          (folds the `mul` branch)
  A[b,z,k]  = ctx[b] @ (lin1_w+lindiff_w).T + (lin1_b + bias + linmul_b
                                               + lindiff_b + lin2_b)
  Bt[b,s,k] = ctx[b] @ (lin2_w-lindiff_w).T              (sans bias; in A's const)

Sharding: V split across the 8 cores (8 k-values per core). Each core:
  phase 1: tmp2[i,(k,z)] = sum_j Wt[j,(k,i)]^T @ ctxT[j,z]   (W-stationary;
           PSUM drained by DVE/ACT copies in parallel)
  phase 2: out[s,(k,z)]  = ctxT[:,s]^T @ tmp2[:,(k,z)], tanh, DMA to a
           (B,S,KV,S) scratch; host transposes/concats.

vs the previous 90.6us version: the 64 fold matmuls (A/Bt/bias add via a
zero-padded 18-row contraction; 20% of PE work) are gone:
  - Bt[s,k] = sum_i ctx[s,i] L2d[k,i] folds into phase 2's dense contraction
    by adding L2d[k,i] to tmp2[i,(k,z)] during the phase-1 PSUM drain
    (tensor_scalar_add / activation-bias: per-partition scalar, free).
  - A[b,(k,z)] is partition-broadcast to a [128, 2048] SBUF tile by an
    otherwise-idle DMA ring (DRAM row read with a stride-0 partition AP),
    then added by DVE in the psum -> f16 output conversion pass; ACT does
    a pure tanh pass after.
PE stream: 128 (phase1) + 128 (phase2) N=512 matmuls at ~220ns warm.
Matmuls run in fp16 (full PE rate); 13 warmup matmuls on a zero tile bridge
engine-init + input-DMA latency so the HAM clock is at K=8/8 when real work
starts. Stores are batched 256KB per psum tile, alternating the sync/gpsimd
rings; the final tile is split in half across both queues to shorten the
tail drain.
"""

import numpy as np

B, S, C, V = 8, 256, 256, 64
NCORES = 8
KV = V // NCORES  # k-values per core
N_WARM = 8        # warmup matmuls on a zero tile (HAM clock ramp bridge)
# Batches whose A-add runs on the PE as a psum-init matmul (ones[1,128] (x)
# A_row, 2 extra N=512 slots per psum tile) instead of a DVE tensor_tensor
# + ACT tanh-from-SBUF. Engine budget: PE slot = 222ns; DVE add 1.22us/tile,
# drains 0.74us, ACT tanh 687ns from PSUM (PE-batches) vs 1148ns from SBUF
# (DVE-batches). P=4 keeps DVE+ACT at ~75% of the PE pace with local slack;
# alternating them in the tail (b5 PE, b6 DVE, b7 PE) stops DVE falling
# behind in the phase2-only stretch.
PE_BATCHES = (0, 1, 5, 6, 7)


def _host_prep(ctx, W, bias, lin1_w, lin1_b, lin2_w, lin2_b,
               linmul_w, linmul_b, lindiff_w, lindiff_b):
    f = np.float32
    ctx = np.asarray(ctx, f)
    Wp = np.array(W, f)
    Wp[np.arange(C), np.arange(C), :] += np.asarray(linmul_w, f).T
    Wt = Wp.transpose(1, 0, 2)  # [j, i, k]

    A = ctx @ (np.asarray(lin1_w, f) + np.asarray(lindiff_w, f)).T \
        + (np.asarray(lin1_b, f) + np.asarray(bias, f) + np.asarray(linmul_b, f)
           + np.asarray(lindiff_b, f) + np.asarray(lin2_b, f))
    L2d = np.asarray(lin2_w, f) - np.asarray(lindiff_w, f)  # [V, C]

    # ctx packed in the phase-1 SBUF tile layout: [pair, c, jchunk, h, z]
    # (h = which batch of the pair): one 2KB-contiguous DMA line per
    # partition -> a single DMA per pair
    ctxT = ctx.transpose(0, 2, 1)  # [B, C, S]
    ctxp = np.ascontiguousarray(
        ctxT.reshape(B // 2, 2, 2, 128, S)      # [pair, h, j, c, z]
            .transpose(0, 3, 2, 1, 4)           # [pair, c, j, h, z]
    ).astype(np.float16)

    # window-major wt packing: [p, (w, j, c in w)] so every window DMA is one
    # contiguous 1-2KB line per partition (128 lines vs 256 x 512B)
    wins = [(0, 256), (256, 768), (768, 1280), (1280, 1792), (1792, 2048)]

    per_core = []
    for c in range(NCORES):
        ks = slice(c * KV, (c + 1) * KV)
        # wt layout: [j*128+p, kk*C + i] -> window-major packed [p, 4096]
        wt0 = Wt[:, :, ks].transpose(0, 2, 1).reshape(2, 128, KV * C)
        wt = np.ascontiguousarray(np.concatenate(
            [np.concatenate([wt0[0][:, lo:hi], wt0[1][:, lo:hi]], axis=1)
             for lo, hi in wins], axis=1)).astype(np.float16)
        # A in (k, z) layout per batch, f16 (broadcast-DMA'd on chip)
        a_c = np.ascontiguousarray(
            A[:, :, ks].transpose(0, 2, 1).reshape(B, KV * S)).astype(np.float16)
        # L2d slice transposed: [i, k] split into two 128-row i-chunks
        l2dT = np.ascontiguousarray(L2d[ks].T.reshape(2, 128, KV))
        per_core.append({"ctxp": ctxp, "wt": wt, "a": a_c, "l2dT": l2dT})
    return per_core


def _build_program():
    import concourse.tile as tile
    import concourse.mybir as mybir
    from concourse import bacc
    from contextlib import ExitStack

    f32 = mybir.dt.float32
    f16 = mybir.dt.float16
    TANH = mybir.ActivationFunctionType.Tanh

    nc = bacc.Bacc("TRN2", target_bir_lowering=False, debug=False)
    ctxp_d = nc.dram_tensor("ctxp", [B // 2, 128, 2 * 2 * S], f16, kind="ExternalInput").ap()
    wt_d = nc.dram_tensor("wt", [128, 2 * KV * C], f16, kind="ExternalInput").ap()
    a_d = nc.dram_tensor("a", [B, KV * S], f16, kind="ExternalInput").ap()
    l2d_d = nc.dram_tensor("l2dT", [2, 128, KV], f32, kind="ExternalInput").ap()
    # out scratch is (k, z)-ordered; the host transposes back to (z, k)
    out_d = nc.dram_tensor("out", [B, S, KV, S], f16, kind="ExternalOutput").ap()

    with tile.TileContext(nc) as tc, ExitStack() as es:
        ctx_pool = es.enter_context(tc.tile_pool(name="ctxp", bufs=8))
        wt_pool = es.enter_context(tc.tile_pool(name="wtp", bufs=2))
        l2d_pool = es.enter_context(tc.tile_pool(name="l2dp", bufs=1))
        arep_pool = es.enter_context(tc.tile_pool(name="arep", bufs=8))
        tmp2_pool = es.enter_context(tc.tile_pool(name="tmp2p", bufs=8))
        ot_pool = es.enter_context(tc.tile_pool(name="otp", bufs=4))
        ot2_pool = es.enter_context(tc.tile_pool(name="ot2p", bufs=6))

        # warmup is emitted first so its wsrc memset leads the DVE queue —
        # the PE can start ramping the HAM clock at engine-init time
        def warmup(ps2_pool):
            wsrc = es.enter_context(tc.tile_pool(name="warmp", bufs=1)).tile(
                [128, 512], f16, name="wsrc", bufs=1)
            nc.vector.memset(wsrc[:], 0.0)
            ones = es.enter_context(tc.tile_pool(name="onesp", bufs=1)).tile(
                [1, 128], f16, name="ones", bufs=1)
            nc.gpsimd.memset(ones[:], 1.0)
            wps = ps2_pool.tile([128, 1024], f32, name="ps2")
            for i in range(N_WARM):
                nc.tensor.matmul(wps[:, (i % 2) * 512:(i % 2) * 512 + 512],
                                 wsrc[:, 0:128], wsrc[:], start=True, stop=True)
            # preload the tanh spline tables while the PE warms up, so the
            # ~1.5us ACT_TABLE_LOAD doesn't stall the first real tanh
            tt = ot2_pool.tile([128, 8], f16, name="ttl", bufs=1)
            nc.scalar.activation(tt[:], wsrc[:, 0:8], TANH)
            return ones

        # Input staging. The sync-engine DMA issue cost is ~0.7us per
        # instruction, so the critical path (ctx pair 0 + progressive wt
        # column windows, consumed kk-major by phase 1) gets the sync ring
        # EXCLUSIVELY, with everything merged into one DMA per pair/window
        # (2KB lines). All bulk/small loads + the arep broadcasts ride the
        # gpsimd ring, queued in need-order.
        ctxp_sb = {}

        def load_ctx_pair(p, eng):
            t = ctx_pool.tile([128, 4 * S], f16, name=f"ctx_{p}", bufs=1)
            eng.dma_start(t[:], ctxp_d[p])
            ctxp_sb[p] = t

        # window-major packed wt: sb col of (j, c) = 2*cum[w] + j*len[w]
        # + (c - cum[w]) where w is the window containing c
        wcum = [0, 256, 768, 1280, 1792, 2048]

        def wt_col(j, c):
            w = next(i for i in range(5) if wcum[i] <= c < wcum[i + 1])
            return 2 * wcum[w] + j * (wcum[w + 1] - wcum[w]) + (c - wcum[w])

        def load_inputs():
            load_ctx_pair(0, nc.sync)
            wt_sb = wt_pool.tile([128, 2 * KV * C], f16, name="wt", bufs=1)
            for w in range(5):
                lo, hi = 2 * wcum[w], 2 * wcum[w + 1]
                nc.sync.dma_start(wt_sb[:, lo:hi], wt_d[:, lo:hi])
            # small loads on the gpsimd ring, in need-order:
            load_ctx_pair(1, nc.gpsimd)
            # L2d bias columns: needed by the first PSUM drain (~11us)
            l2d_sb = l2d_pool.tile([128, 2 * KV], f32, name="l2dT", bufs=1)
            nc.gpsimd.dma_start(
                l2d_sb[:].rearrange("p (ch k) -> p ch k", ch=2),
                l2d_d.rearrange("ch p k -> p ch k"))
            # A rows for the PE-init batches: [1, 2048] f16, tiny DMAs
            arow = {}
            for b in PE_BATCHES:
                t = arep_pool.tile([1, KV * S], f16, name=f"arow_{b}", bufs=1)
                nc.gpsimd.dma_start(t[:], a_d[b:b + 1])
                arow[b] = t
            # bulk transfers (ctx pairs 2/3, A broadcasts) also ride the
            # gpsimd ring: its ~0.65us per-DMA issue cadence throttles them
            # naturally, so they trickle in behind the critical sync-ring
            # loads; none of them is needed before ~30us.
            load_ctx_pair(2, nc.gpsimd)
            load_ctx_pair(3, nc.gpsimd)
            # A[b] broadcast to all 128 partitions straight from DRAM
            # (stride-0 partition AP) for the DVE-add batches; 512KB each.
            arep = {}
            for b in range(B):
                if b in PE_BATCHES:
                    continue
                t = arep_pool.tile([128, KV * S], f16, name=f"arep_{b}", bufs=1)
                nc.gpsimd.dma_start(t[:], a_d[b:b + 1].to_broadcast([128, KV * S]))
                arep[b] = t
            return wt_sb, l2d_sb, arow, arep

        tmp2p = {}

        def phase1(pg, ps1_pool, copy_engines=("vector",), chs=(0, 1)):
            # kk-major so the wt columns are consumed left-to-right, matching
            # the progressive wt window DMAs
            ce = [0]
            for ch in chs:
                for p in pg:
                    tmp2p[p, ch] = tmp2_pool.tile([128, 2 * KV * S], f16, name="tmp2")
            for kk in range(KV):
                for ch in chs:  # i-chunk (output partition of tmp2)
                    ps = {}
                    for p in pg:
                        ps[p] = ps1_pool.tile([128, 2 * S], f32, name="ps1")
                    for j in range(2):  # contraction chunk
                        off = wt_col(j, kk * C + ch * 128)
                        lhsT = wt_sb[:, off: off + 128]
                        for p in pg:
                            nc.tensor.matmul(
                                ps[p][:], lhsT,
                                ctxp_sb[p][:, j * 2 * S:(j + 1) * 2 * S],
                                start=(j == 0), stop=(j == 1),
                            )
                    bias_ap = l2d_sb[:, ch * KV + kk: ch * KV + kk + 1]
                    for p in pg:
                        # drain + fold Bt: tmp2[i,(h,kk,z)] = psum + L2d[kk,i]
                        dst = tmp2p[p, ch][:].rearrange("q (h k z) -> q h k z", h=2, k=KV)
                        src_ap = ps[p][:].rearrange("q (h z) -> q h z", h=2)
                        eng = copy_engines[ce[0] % len(copy_engines)]
                        ce[0] += 1
                        if eng == "vector":
                            nc.vector.tensor_scalar_add(dst[:, :, kk, :], src_ap, bias_ap)
                        else:
                            nc.scalar.add(dst[:, :, kk, :], src_ap, bias_ap)

        st_ctr = [0]

        def phase2(bg, ps2_pool, split_store=False):
            for b in bg:
                pe_init = b in PE_BATCHES
                for sc in range(2):
                    hoff = (b % 2) * KV * S
                    for t in range(2):  # double-bank psum tiles, 2 n-chunks each
                        pst = ps2_pool.tile([128, 1024], f32, name="ps2")
                        n0 = 2 * t
                        if pe_init:
                            # psum = A broadcast over the s partitions
                            # (ones[1,128] (x) A_row[1,512], K=1 matmuls;
                            # one per psum bank)
                            for n in (n0, n0 + 1):
                                nc.tensor.matmul(
                                    pst[:, (n % 2) * 512:(n % 2) * 512 + 512],
                                    ones_sb[:],
                                    arow[b][:, n * 512:(n + 1) * 512],
                                    start=True, stop=False)
                        for st in range(2):  # contraction chunk; one LDW per 2 MMs
                            soff = st * 2 * S + (b % 2) * S + sc * 128
                            lhsT = ctxp_sb[b // 2][:, soff: soff + 128]
                            for n in (n0, n0 + 1):
                                nc.tensor.matmul(
                                    pst[:, (n % 2) * 512:(n % 2) * 512 + 512], lhsT,
                                    tmp2p[b // 2, st][:, hoff + n * 512:hoff + (n + 1) * 512],
                                    start=False if pe_init else (st == 0),
                                    stop=(st == 1),
                                )
                        if pe_init:
                            tanh_src = pst[:]
                        else:
                            # DVE: psum + A (broadcast tile) -> f32 staging
                            ot = ot_pool.tile([128, 1024], f32, name="ot")
                            nc.vector.tensor_add(ot[:], pst[:],
                                                 arep[b][:, t * 1024:(t + 1) * 1024])
                            tanh_src = ot[:]
                        ot2 = ot2_pool.tile([128, 1024], f16, name="ot2")
                        if split_store and sc == 1 and t == 1:
                            # very last tile: split tanh + store per psum bank so
                            # each half-store starts as soon as its half is done,
                            # halved across two queues for the shortest drain
                            # (sync + scalar; the gpsimd queue tends to carry
                            # a backlog at this point)
                            for hd in range(2):
                                nc.scalar.activation(ot2[:, hd * 512:(hd + 1) * 512],
                                                     tanh_src[:, hd * 512:(hd + 1) * 512],
                                                     TANH)
                                eng = nc.sync if hd == 0 else nc.scalar
                                eng.dma_start(
                                    out_d[b, sc * 128:(sc + 1) * 128,
                                          4 + 2 * hd:6 + 2 * hd]
                                    .rearrange("s k z -> s (k z)"),
                                    ot2[:, hd * 512:(hd + 1) * 512],
                                )
                        else:
                            nc.scalar.activation(ot2[:], tanh_src, TANH)
                            # stripe stores across the two HWDGE rings only
                            # (SWDGE/gpsimd store transfers drain slowly and
                            # stretch the end-of-kernel queue drain)
                            eng = (nc.sync, nc.scalar)[st_ctr[0] % 2]
                            st_ctr[0] += 1
                            eng.dma_start(
                                out_d[b, sc * 128:(sc + 1) * 128, 4 * t:4 * t + 4]
                                .rearrange("s k z -> s (k z)"),
                                ot2[:],
                            )

        ps1_pool = es.enter_context(tc.tile_pool(name="ps1", bufs=4, space="PSUM"))
        ps2_pool = es.enter_context(tc.tile_pool(name="ps2", bufs=2, space="PSUM"))
        # drains ~2:1 on DVE (ACT carries the tanh stream)
        mix = ("vector", "vector", "scalar")
        ones_sb = warmup(ps2_pool)
        wt_sb, l2d_sb, arow, arep = load_inputs()
        phase1([0], ps1_pool, copy_engines=mix)
        phase2([0], ps2_pool)
        phase1([1], ps1_pool, copy_engines=mix)
        phase2([1], ps2_pool)
        phase2([2], ps2_pool)
        phase1([2], ps1_pool, copy_engines=mix)
        phase2([3], ps2_pool)
        phase1([3], ps1_pool, copy_engines=("vector", "scalar"), chs=(0,))
        phase2([4], ps2_pool)
        phase1([3], ps1_pool, copy_engines=("vector", "scalar"), chs=(1,))
        phase2([5], ps2_pool)
        phase2([6], ps2_pool)
        phase2([7], ps2_pool, split_store=True)

    nc.compile()
    return nc


def _install_profile_hook():
    """Register the NTFF profile hook that the image's boot skipped
    (antenv.axon_hooks shim is missing in this container)."""
    import sys as _sys
    import types as _types
    try:
        import antenv
        if "antenv.axon_hooks" not in _sys.modules:
            m = _types.ModuleType("antenv.axon_hooks")
            _h = [None]
            m.set_axon_ntff_profile_hook = lambda h: _h.__setitem__(0, h)
            m.get_axon_ntff_profile_hook = lambda: _h[0]
            _sys.modules["antenv.axon_hooks"] = m
            antenv.axon_hooks = m
        from antenv.axon_hooks import set_axon_ntff_profile_hook, get_axon_ntff_profile_hook
        if get_axon_ntff_profile_hook() is None:
            from trn_agent_boot.trn_boot import _ntff_profile_via_ctypes
            set_axon_ntff_profile_hook(_ntff_profile_via_ctypes("/opt/axon/libaxon_pjrt.so"))
    except Exception:
        pass


def run(inputs, trace=False, repeats=1):
    """Returns (full_output, BassKernelResults)."""
    from concourse.bass_utils import run_bass_kernel_spmd

    if trace:
        _install_profile_hook()
    per_core = _host_prep(**inputs)
    nc = _build_program()
    import os as _os
    _tc = [int(x) for x in _os.environ.get("KERNEL_TRACE_CORES", "0").split(",")]
    times = []
    for r in range(repeats):
        res = run_bass_kernel_spmd(nc, per_core, list(range(NCORES)), trace=trace,
                                   trace_cores=_tc if trace else None)
        if res.exec_time_ns is not None:
            times.append(res.exec_time_ns)
    if times:
        res.all_exec_times_ns = times
    # per-core scratch is (B, S, KV, S) with k-major planes: swap to (B,S,S,KV)
    out = np.concatenate(
        [res.results[c]["out"].astype(np.float32).transpose(0, 1, 3, 2)
         for c in range(NCORES)], axis=3)
    out = np.ascontiguousarray(out)
    return out, res


def kernel(**inputs) -> np.ndarray:
    out, _ = run(inputs, trace=False)
    return out


# revision 40
# speedup vs baseline: 1.0141x; 1.0141x over previous
"""Trainium2 Bass kernel for nn_BilinAndFwdComboVecComp.

Math (B=8, S=256, C=256, V=64):
  final[b,s,z,k] = tanh( sum_ij ctx[b,s,i] ctx[b,z,j] W'[i,j,k] + A[b,z,k] + Bt[b,s,k] )
where
  W'[i,j,k] = W[i,j,k] + (i==j) * linmul_w[k,i]          (folds the `mul` branch)
  A[b,z,k]  = ctx[b] @ (lin1_w+lindiff_w).T + (lin1_b + bias + linmul_b
                                               + lindiff_b + lin2_b)
  Bt[b,s,k] = ctx[b] @ (lin2_w-lindiff_w).T              (sans bias; in A's const)

Sharding: V split across the 8 cores (8 k-values per core). Each core:
  phase 1: tmp2[i,(k,z)] = sum_j Wt[j,(k,i)]^T @ ctxT[j,z]   (W-stationary;
           PSUM drained by DVE/ACT copies in parallel)
  phase 2: out[s,(k,z)]  = ctxT[:,s]^T @ tmp2[:,(k,z)], tanh, DMA to a
           (B,S,KV,S) scratch; host transposes/concats.

vs the previous 90.6us version: the 64 fold matmuls (A/Bt/bias add via a
zero-padded 18-row contraction; 20% of PE work) are gone:
  - Bt[s,k] = sum_i ctx[s,i] L2d[k,i] folds into phase 2's dense contraction
    by adding L2d[k,i] to tmp2[i,(k,z)] during the phase-1 PSUM drain
    (tensor_scalar_add / activation-bias: per-partition scalar, free).
  - A[b,(k,z)] is partition-broadcast to a [128, 2048] SBUF tile by an
    otherwise-idle DMA ring (DRAM row read with a stride-0 partition AP),
    then added by DVE in the psum -> f16 output conversion pass; ACT does
    a pure tanh pass after.
PE stream: 128 (phase1) + 128 (phase2) N=512 matmuls at ~220ns warm.
Matmuls run in fp16 (full PE rate); 13 warmup matmuls on a zero tile bridge
engine-init + input-DMA latency so the HAM clock is at K=8/8 when real work
starts. Stores are batched 256KB per psum tile, alternating the sync/gpsimd
rings; the final tile is split in half across both queues to shorten the
tail drain.
"""

import numpy as np

B, S, C, V = 8, 256, 256, 64
NCORES = 8
KV = V // NCORES  # k-values per core
N_WARM = 7        # warmup matmuls on a zero tile (HAM clock ramp bridge)
# Batches whose A-add runs on the PE as a psum-init matmul (ones[1,128] (x)
# A_row, 2 extra N=512 slots per psum tile) instead of a DVE tensor_tensor
# + ACT tanh-from-SBUF. Engine budget: PE slot = 222ns; DVE add 1.22us/tile,
# drains 0.74us, ACT tanh 687ns from PSUM (PE-batches) vs 1148ns from SBUF
# (DVE-batches). P=4 keeps DVE+ACT at ~75% of the PE pace with local slack;
# alternating them in the tail (b5 PE, b6 DVE, b7 PE) stops DVE falling
# behind in the phase2-only stretch.
PE_BATCHES = (0, 1, 5, 7)


def _host_prep(ctx, W, bias, lin1_w, lin1_b, lin2_w, lin2_b,
               linmul_w, linmul_b, lindiff_w, lindiff_b):
    f = np.float32
    ctx = np.asarray(ctx, f)
    Wp = np.array(W, f)
    Wp[np.arange(C), np.arange(C), :] += np.asarray(linmul_w, f).T
    Wt = Wp.transpose(1, 0, 2)  # [j, i, k]

    A = ctx @ (np.asarray(lin1_w, f) + np.asarray(lindiff_w, f)).T \
        + (np.asarray(lin1_b, f) + np.asarray(bias, f) + np.asarray(linmul_b, f)
           + np.asarray(lindiff_b, f) + np.asarray(lin2_b, f))
    L2d = np.asarray(lin2_w, f) - np.asarray(lindiff_w, f)  # [V, C]

    # ctx packed in the phase-1 SBUF tile layout: [pair, c, jchunk, h, z]
    # (h = which batch of the pair): one 2KB-contiguous DMA line per
    # partition -> a single DMA per pair
    ctxT = ctx.transpose(0, 2, 1)  # [B, C, S]
    ctxp = np.ascontiguousarray(
        ctxT.reshape(B // 2, 2, 2, 128, S)      # [pair, h, j, c, z]
            .transpose(0, 3, 2, 1, 4)           # [pair, c, j, h, z]
    ).astype(np.float16)

    # window-major wt packing: [p, (w, j, c in w)] so every window DMA is one
    # contiguous 1-2KB line per partition (128 lines vs 256 x 512B)
    wins = [(0, 256), (256, 768), (768, 1280), (1280, 1792), (1792, 2048)]

    per_core = []
    for c in range(NCORES):
        ks = slice(c * KV, (c + 1) * KV)
        # wt layout: [j*128+p, kk*C + i] -> window-major packed [p, 4096]
        wt0 = Wt[:, :, ks].transpose(0, 2, 1).reshape(2, 128, KV * C)
        wt = np.ascontiguousarray(np.concatenate(
            [np.concatenate([wt0[0][:, lo:hi], wt0[1][:, lo:hi]], axis=1)
             for lo, hi in wins], axis=1)).astype(np.float16)
        # A in (k, z) layout per batch, f16 (broadcast-DMA'd on chip)
        a_c = np.ascontiguousarray(
            A[:, :, ks].transpose(0, 2, 1).reshape(B, KV * S)).astype(np.float16)
        # L2d slice transposed: [i, k] split into two 128-row i-chunks
        l2dT = np.ascontiguousarray(L2d[ks].T.reshape(2, 128, KV))
        per_core.append({"ctxp": ctxp, "wt": wt, "a": a_c, "l2dT": l2dT})
    return per_core


def _build_program():
    import concourse.tile as tile
    import concourse.mybir as mybir
    from concourse import bacc
    from contextlib import ExitStack

    f32 = mybir.dt.float32
    f16 = mybir.dt.float16
    TANH = mybir.ActivationFunctionType.Tanh

    nc = bacc.Bacc("TRN2", target_bir_lowering=False, debug=False)
    ctxp_d = nc.dram_tensor("ctxp", [B // 2, 128, 2 * 2 * S], f16, kind="ExternalInput").ap()
    wt_d = nc.dram_tensor("wt", [128, 2 * KV * C], f16, kind="ExternalInput").ap()
    a_d = nc.dram_tensor("a", [B, KV * S], f16, kind="ExternalInput").ap()
    l2d_d = nc.dram_tensor("l2dT", [2, 128, KV], f32, kind="ExternalInput").ap()
    # out scratch is (k, z)-ordered; the host transposes back to (z, k)
    out_d = nc.dram_tensor("out", [B, S, KV, S], f16, kind="ExternalOutput").ap()

    with tile.TileContext(nc) as tc, ExitStack() as es:
        ctx_pool = es.enter_context(tc.tile_pool(name="ctxp", bufs=8))
        wt_pool = es.enter_context(tc.tile_pool(name="wtp", bufs=2))
        l2d_pool = es.enter_context(tc.tile_pool(name="l2dp", bufs=1))
        arep_pool = es.enter_context(tc.tile_pool(name="arep", bufs=8))
        tmp2_pool = es.enter_context(tc.tile_pool(name="tmp2p", bufs=8))
        ot_pool = es.enter_context(tc.tile_pool(name="otp", bufs=4))
        ot2_pool = es.enter_context(tc.tile_pool(name="ot2p", bufs=6))

        # warmup is emitted first so its wsrc memset leads the DVE queue —
        # the PE can start ramping the HAM clock at engine-init time
        def warmup(ps2_pool):
            wsrc = es.enter_context(tc.tile_pool(name="warmp", bufs=1)).tile(
                [128, 512], f16, name="wsrc", bufs=1)
            nc.vector.memset(wsrc[:], 0.0)
            ones = es.enter_context(tc.tile_pool(name="onesp", bufs=1)).tile(
                [1, 128], f16, name="ones", bufs=1)
            nc.gpsimd.memset(ones[:], 1.0)
            wps = ps2_pool.tile([128, 1024], f32, name="ps2")
            for i in range(N_WARM):
                nc.tensor.matmul(wps[:, (i % 2) * 512:(i % 2) * 512 + 512],
                                 wsrc[:, 0:128], wsrc[:], start=True, stop=True)
            # preload the tanh spline tables while the PE warms up, so the
            # ~1.5us ACT_TABLE_LOAD doesn't stall the first real tanh
            tt = ot2_pool.tile([128, 8], f16, name="ttl", bufs=1)
            nc.scalar.activation(tt[:], wsrc[:, 0:8], TANH)
            return ones

        # Input staging. The sync-engine DMA issue cost is ~0.7us per
        # instruction, so the critical path (ctx pair 0 + progressive wt
        # column windows, consumed kk-major by phase 1) gets the sync ring
        # EXCLUSIVELY, with everything merged into one DMA per pair/window
        # (2KB lines). All bulk/small loads + the arep broadcasts ride the
        # gpsimd ring, queued in need-order.
        ctxp_sb = {}

        def load_ctx_pair(p, eng):
            t = ctx_pool.tile([128, 4 * S], f16, name=f"ctx_{p}", bufs=1)
            eng.dma_start(t[:], ctxp_d[p])
            ctxp_sb[p] = t

        # window-major packed wt: sb col of (j, c) = 2*cum[w] + j*len[w]
        # + (c - cum[w]) where w is the window containing c
        wcum = [0, 256, 768, 1280, 1792, 2048]

        def wt_col(j, c):
            w = next(i for i in range(5) if wcum[i] <= c < wcum[i + 1])
            return 2 * wcum[w] + j * (wcum[w + 1] - wcum[w]) + (c - wcum[w])

        def load_inputs():
            load_ctx_pair(0, nc.sync)
            wt_sb = wt_pool.tile([128, 2 * KV * C], f16, name="wt", bufs=1)
            for w in range(5):
                lo, hi = 2 * wcum[w], 2 * wcum[w + 1]
                nc.sync.dma_start(wt_sb[:, lo:hi], wt_d[:, lo:hi])
            # small loads on the gpsimd ring, in need-order:
            load_ctx_pair(1, nc.gpsimd)
            # L2d bias columns: needed by the first PSUM drain (~11us)
            l2d_sb = l2d_pool.tile([128, 2 * KV], f32, name="l2dT", bufs=1)
            nc.gpsimd.dma_start(
                l2d_sb[:].rearrange("p (ch k) -> p ch k", ch=2),
                l2d_d.rearrange("ch p k -> p ch k"))
            # A rows for the PE-init batches: [1, 2048] f16, tiny DMAs
            arow = {}
            for b in PE_BATCHES:
                t = arep_pool.tile([1, KV * S], f16, name=f"arow_{b}", bufs=1)
                nc.gpsimd.dma_start(t[:], a_d[b:b + 1])
                arow[b] = t
            # bulk transfers (ctx pairs 2/3, A broadcasts) also ride the
            # gpsimd ring: its ~0.65us per-DMA issue cadence throttles them
            # naturally, so they trickle in behind the critical sync-ring
            # loads; none of them is needed before ~30us.
            load_ctx_pair(2, nc.gpsimd)
            load_ctx_pair(3, nc.gpsimd)
            # A[b] broadcast to all 128 partitions straight from DRAM
            # (stride-0 partition AP) for the DVE-add batches; 512KB each.
            arep = {}
            for b in range(B):
                if b in PE_BATCHES:
                    continue
                t = arep_pool.tile([128, KV * S], f16, name=f"arep_{b}", bufs=1)
                nc.gpsimd.dma_start(t[:], a_d[b:b + 1].to_broadcast([128, KV * S]))
                arep[b] = t
            return wt_sb, l2d_sb, arow, arep

        tmp2p = {}

        def phase1(pg, ps1_pool, copy_engines=("vector",), chs=(0, 1)):
            # kk-major so the wt columns are consumed left-to-right, matching
            # the progressive wt window DMAs
            ce = [0]
            for ch in chs:
                for p in pg:
                    tmp2p[p, ch] = tmp2_pool.tile([128, 2 * KV * S], f16, name="tmp2")
            for kk in range(KV):
                for ch in chs:  # i-chunk (output partition of tmp2)
                    ps = {}
                    for p in pg:
                        ps[p] = ps1_pool.tile([128, 2 * S], f32, name="ps1")
                    for j in range(2):  # contraction chunk
                        off = wt_col(j, kk * C + ch * 128)
                        lhsT = wt_sb[:, off: off + 128]
                        for p in pg:
                            nc.tensor.matmul(
                                ps[p][:], lhsT,
                                ctxp_sb[p][:, j * 2 * S:(j + 1) * 2 * S],
                                start=(j == 0), stop=(j == 1),
                            )
                    bias_ap = l2d_sb[:, ch * KV + kk: ch * KV + kk + 1]
                    for p in pg:
                        # drain + fold Bt: tmp2[i,(h,kk,z)] = psum + L2d[kk,i]
                        dst = tmp2p[p, ch][:].rearrange("q (h k z) -> q h k z", h=2, k=KV)
                        src_ap = ps[p][:].rearrange("q (h z) -> q h z", h=2)
                        eng = copy_engines[ce[0] % len(copy_engines)]
                        ce[0] += 1
                        if eng == "vector":
                            nc.vector.tensor_scalar_add(dst[:, :, kk, :], src_ap, bias_ap)
                        else:
                            nc.scalar.add(dst[:, :, kk, :], src_ap, bias_ap)

        st_ctr = [0]

        def phase2(bg, ps2_pool, split_store=False):
            for b in bg:
                pe_init = b in PE_BATCHES
                for sc in range(2):
                    hoff = (b % 2) * KV * S
                    for t in range(2):  # double-bank psum tiles, 2 n-chunks each
                        pst = ps2_pool.tile([128, 1024], f32, name="ps2")
                        n0 = 2 * t
                        if pe_init:
                            # psum = A broadcast over the s partitions
                            # (ones[1,128] (x) A_row[1,512], K=1 matmuls;
                            # one per psum bank)
                            for n in (n0, n0 + 1):
                                nc.tensor.matmul(
                                    pst[:, (n % 2) * 512:(n % 2) * 512 + 512],
                                    ones_sb[:],
                                    arow[b][:, n * 512:(n + 1) * 512],
                                    start=True, stop=False)
                        for st in range(2):  # contraction chunk; one LDW per 2 MMs
                            soff = st * 2 * S + (b % 2) * S + sc * 128
                            lhsT = ctxp_sb[b // 2][:, soff: soff + 128]
                            for n in (n0, n0 + 1):
                                nc.tensor.matmul(
                                    pst[:, (n % 2) * 512:(n % 2) * 512 + 512], lhsT,
                                    tmp2p[b // 2, st][:, hoff + n * 512:hoff + (n + 1) * 512],
                                    start=False if pe_init else (st == 0),
                                    stop=(st == 1),
                                )
                        if pe_init:
                            tanh_src = pst[:]
                        else:
                            # DVE: psum + A (broadcast tile) -> f32 staging
                            ot = ot_pool.tile([128, 1024], f32, name="ot")
                            nc.vector.tensor_add(ot[:], pst[:],
                                                 arep[b][:, t * 1024:(t + 1) * 1024])
                            tanh_src = ot[:]
                        ot2 = ot2_pool.tile([128, 1024], f16, name="ot2")
                        if split_store and sc == 1 and t == 1:
                            # very last tile: split tanh + store per psum bank so
                            # each half-store starts as soon as its half is done,
                            # halved across two queues for the shortest drain
                            # (sync + scalar; the gpsimd queue tends to carry
                            # a backlog at this point)
                            for hd in range(2):
                                nc.scalar.activation(ot2[:, hd * 512:(hd + 1) * 512],
                                                     tanh_src[:, hd * 512:(hd + 1) * 512],
                                                     TANH)
                                eng = nc.sync if hd == 0 else nc.scalar
                                eng.dma_start(
                                    out_d[b, sc * 128:(sc + 1) * 128,
                                          4 + 2 * hd:6 + 2 * hd]
                                    .rearrange("s k z -> s (k z)"),
                                    ot2[:, hd * 512:(hd + 1) * 512],
                                )
                        else:
                            nc.scalar.activation(ot2[:], tanh_src, TANH)
                            # stripe stores across the two HWDGE rings only
                            # (SWDGE/gpsimd store transfers drain slowly and
                            # stretch the end-of-kernel queue drain)
                            eng = (nc.sync, nc.scalar)[st_ctr[0] % 2]
                            st_ctr[0] += 1
                            eng.dma_start(
                                out_d[b, sc * 128:(sc + 1) * 128, 4 * t:4 * t + 4]
                                .rearrange("s k z -> s (k z)"),
                                ot2[:],
                            )

        ps1_pool = es.enter_context(tc.tile_pool(name="ps1", bufs=4, space="PSUM"))
        ps2_pool = es.enter_context(tc.tile_pool(name="ps2", bufs=2, space="PSUM"))
        # drains ~2:1 on DVE (ACT carries the tanh stream)
        mix = ("vector", "vector", "scalar")
        ones_sb = warmup(ps2_pool)
        wt_sb, l2d_sb, arow, arep = load_inputs()
        phase1([0], ps1_pool, copy_engines=mix)
        phase2([0], ps2_pool)
        phase1([1], ps1_pool, copy_engines=mix)
        phase2([1], ps2_pool)
        phase2([2], ps2_pool)
        phase1([2], ps1_pool, copy_engines=mix)
        phase2([3], ps2_pool)
        phase1([3], ps1_pool, copy_engines=("vector", "scalar"), chs=(0,))
        phase2([4], ps2_pool)
        phase1([3], ps1_pool, copy_engines=("vector", "scalar"), chs=(1,))
        # b6 (DVE-add batch) before the PE-init b5/b7 so the final tiles'
        # tanhs read PSUM (687ns) with ACT otherwise idle - shortest tail
        phase2([6], ps2_pool)
        phase2([5], ps2_pool)
        phase2([7], ps2_pool, split_store=True)

    nc.compile()
    return nc


def _install_profile_hook():
    """Register the NTFF profile hook that the image's boot skipped
    (antenv.axon_hooks shim is missing in this container)."""
    import sys as _sys
    import types as _types
    try:
        import antenv
        if "antenv.axon_hooks" not in _sys.modules:
            m = _types.ModuleType("antenv.axon_hooks")
            _h = [None]
            m.set_axon_ntff_profile_hook = lambda h: _h.__setitem__(0, h)
            m.get_axon_ntff_profile_hook = lambda: _h[0]
            _sys.modules["antenv.axon_hooks"] = m
            antenv.axon_hooks = m
        from antenv.axon_hooks import set_axon_ntff_profile_hook, get_axon_ntff_profile_hook
        if get_axon_ntff_profile_hook() is None:
            from trn_agent_boot.trn_boot import _ntff_profile_via_ctypes
            set_axon_ntff_profile_hook(_ntff_profile_via_ctypes("/opt/axon/libaxon_pjrt.so"))
    except Exception:
        pass


def run(inputs, trace=False, repeats=1):
    """Returns (full_output, BassKernelResults)."""
    from concourse.bass_utils import run_bass_kernel_spmd

    if trace:
        _install_profile_hook()
    per_core = _host_prep(**inputs)
    nc = _build_program()
    import os as _os
    _tc = [int(x) for x in _os.environ.get("KERNEL_TRACE_CORES", "0").split(",")]
    times = []
    for r in range(repeats):
        res = run_bass_kernel_spmd(nc, per_core, list(range(NCORES)), trace=trace,
                                   trace_cores=_tc if trace else None)
        if res.exec_time_ns is not None:
            times.append(res.exec_time_ns)
    if times:
        res.all_exec_times_ns = times
    # per-core scratch is (B, S, KV, S) with k-major planes: swap to (B,S,S,KV)
    out = np.concatenate(
        [res.results[c]["out"].astype(np.float32).transpose(0, 1, 3, 2)
         for c in range(NCORES)], axis=3)
    out = np.ascontiguousarray(out)
    return out, res


def kernel(**inputs) -> np.ndarray:
    out, _ = run(inputs, trace=False)
    return out
